# revision 6
# baseline (speedup 1.0000x reference)
"""AttnBlock (GroupNorm -> 1x1-conv QKV self-attention -> 1x1-conv out -> residual)
for Trainium2, data-parallel over batch across 8 NeuronCores.

Contract: kernel(**inputs) takes the FULL inputs (np arrays, dtypes as in
setup_inputs) and returns the FULL output [32, 256, 32, 32] fp32.

Strategy (per core, 4 batches):
  - GroupNorm stats via per-channel bn_stats/bn_aggr + tiny group-reduce /
    broadcast matmuls with constant indicator matrices (host-provided).
  - wo is folded into wv on the host (out = x + (wo@wv) h attn + wo@bv + bo),
    so the device does 3 projections, scores, softmax, one attention matmul
    and a PE-transpose epilogue.
  - Softmax denominators come free from a ones-column appended to the v'
    tiles (softmax rows sum the exp'd scores); no max-subtraction is needed
    (|scores| <= ~9 for this problem regime, exp is fp32-safe).
  - All big matmuls run as float32r (full PE speed at free-dim >= 256).
"""
import numpy as np

import concourse.bacc as bacc
import concourse.mybir as mybir
import concourse.tile as tile
from concourse.bass_utils import run_bass_kernel_spmd

N_CORES = 8
B, C, H, W = 32, 256, 32, 32
NSP = H * W            # 1024 spatial positions
BL = B // N_CORES      # 4 batches per core
CT = C // 128          # 2 channel tiles of 128
NGT = 16               # groups per channel tile
GS = 8                 # channels per group
EPS = 1e-5
SM_SCALE = 1.0 / 16.0  # C ** -0.5
F32 = mybir.dt.float32
F32R = mybir.dt.float32r

_CACHE: dict = {}


def _build():
    nc = bacc.Bacc(None, target_bir_lowering=False)

    x_d = nc.dram_tensor("x", [BL, C, NSP], F32, kind="ExternalInput")
    wqT_d = nc.dram_tensor("wqT", [C, C], F32, kind="ExternalInput")
    wkT_d = nc.dram_tensor("wkT", [C, C], F32, kind="ExternalInput")
    wvpT_d = nc.dram_tensor("wvpT", [C, C], F32, kind="ExternalInput")
    bq_d = nc.dram_tensor("bq", [C], F32, kind="ExternalInput")
    bk_d = nc.dram_tensor("bk", [C], F32, kind="ExternalInput")
    bop_d = nc.dram_tensor("bop", [C], F32, kind="ExternalInput")
    gnsc_d = nc.dram_tensor("gnsc", [C], F32, kind="ExternalInput")
    gnbi_d = nc.dram_tensor("gnbi", [C], F32, kind="ExternalInput")
    a_d = nc.dram_tensor("gmatA", [128, NGT], F32, kind="ExternalInput")
    at_d = nc.dram_tensor("gmatAT", [NGT, 128], F32, kind="ExternalInput")
    id_d = nc.dram_tensor("ident", [128, 128], F32, kind="ExternalInput")
    out_d = nc.dram_tensor("out", [BL, C, NSP], F32, kind="ExternalOutput")

    with tile.TileContext(nc) as tc:
        with tc.tile_pool(name="consts", bufs=1) as consts, \
             tc.tile_pool(name="xp", bufs=2) as xp, \
             tc.tile_pool(name="hp", bufs=2) as hp, \
             tc.tile_pool(name="qk", bufs=1) as qk, \
             tc.tile_pool(name="vp", bufs=8) as vp, \
             tc.tile_pool(name="ep", bufs=8) as ep, \
             tc.tile_pool(name="op", bufs=8) as op, \
             tc.tile_pool(name="outp", bufs=2) as outp, \
             tc.tile_pool(name="small", bufs=6) as small, \
             tc.tile_pool(name="ps", bufs=8, space="PSUM") as ps:

            # ---- constants (loaded once, reused for all 4 batches) ----
            wqT = consts.tile([128, CT, C], F32R, tag="wqT")
            nc.sync.dma_start(out=wqT, in_=wqT_d.rearrange("(t p) o -> p t o", p=128).bitcast(F32R))
            wkT = consts.tile([128, CT, C], F32R, tag="wkT")
            nc.sync.dma_start(out=wkT, in_=wkT_d.rearrange("(t p) o -> p t o", p=128).bitcast(F32R))
            wvpT = consts.tile([128, CT, C], F32R, tag="wvpT")
            nc.sync.dma_start(out=wvpT, in_=wvpT_d.rearrange("(t p) o -> p t o", p=128).bitcast(F32R))
            bq = consts.tile([128, CT], F32, tag="bq")
            nc.sync.dma_start(out=bq, in_=bq_d.rearrange("(t p) -> p t", p=128))
            bk = consts.tile([128, CT], F32, tag="bk")
            nc.sync.dma_start(out=bk, in_=bk_d.rearrange("(t p) -> p t", p=128))
            bop = consts.tile([128, CT], F32, tag="bop")
            nc.sync.dma_start(out=bop, in_=bop_d.rearrange("(t p) -> p t", p=128))
            gnsc = consts.tile([128, CT], F32, tag="gnsc")
            nc.sync.dma_start(out=gnsc, in_=gnsc_d.rearrange("(t p) -> p t", p=128))
            gnbi = consts.tile([128, CT], F32, tag="gnbi")
            nc.sync.dma_start(out=gnbi, in_=gnbi_d.rearrange("(t p) -> p t", p=128))
            gA = consts.tile([128, NGT], F32, tag="gA")
            nc.sync.dma_start(out=gA, in_=a_d[:, :])
            gAT = consts.tile([NGT, 128], F32, tag="gAT")
            nc.sync.dma_start(out=gAT, in_=at_d[:, :])
            ident = consts.tile([128, 128], F32, tag="ident")
            nc.sync.dma_start(out=ident, in_=id_d[:, :])
            ones = consts.tile([128, 2], F32, tag="ones")
            nc.vector.memset(ones, 1.0)
            eps_sb = consts.tile([128, 1], F32, tag="eps")
            nc.vector.memset(eps_sb, EPS)

            for b in range(BL):
                # ---- load x[b] as [128 part, 2 ctile, 1024 spatial] ----
                x_sb = xp.tile([128, CT, NSP], F32, tag="x")
                nc.sync.dma_start(out=x_sb, in_=x_d[b].rearrange("(t p) n -> p t n", p=128))

                # ---- GroupNorm -> h (fp32r) ----
                h_sb = hp.tile([128, CT, NSP], F32R, tag="h")
                for t in range(CT):
                    st = small.tile([128, 2, 6], F32, tag="bnst")
                    nc.vector.bn_stats(out=st[:, 0, :], in_=x_sb[:, t, 0:512])
                    nc.vector.bn_stats(out=st[:, 1, :], in_=x_sb[:, t, 512:1024])
                    mv = small.tile([128, 2], F32, tag="mv")
                    nc.vector.bn_aggr(out=mv, in_=st)
                    # mm_in = [mean_c, var_c + mean_c^2]
                    mm_in = small.tile([128, 2], F32, tag="mmin")
                    nc.vector.tensor_copy(out=mm_in[:, 0:1], in_=mv[:, 0:1])
                    nc.vector.tensor_mul(mm_in[:, 1:2], mv[:, 0:1], mv[:, 0:1])
                    nc.vector.tensor_add(mm_in[:, 1:2], mm_in[:, 1:2], mv[:, 1:2])
                    # per-group sums over the 8 channels of each group
                    gps = ps.tile([128, 512], F32, tag="ps")
                    nc.tensor.matmul(gps[:NGT, :2], gA, mm_in, start=True, stop=True)
                    # group stats: gst = [mean_g, rstd_g]
                    gst = small.tile([NGT, 2], F32, tag="gst")
                    nc.scalar.mul(out=gst, in_=gps[:NGT, :2], mul=1.0 / GS)
                    tmp = small.tile([NGT, 2], F32, tag="gtmp")
                    nc.vector.tensor_mul(tmp[:, 0:1], gst[:, 0:1], gst[:, 0:1])
                    nc.vector.tensor_tensor(tmp[:, 1:2], gst[:, 1:2], tmp[:, 0:1],
                                            mybir.AluOpType.subtract)
                    nc.scalar.activation(out=tmp[:, 1:2], in_=tmp[:, 1:2],
                                         func=mybir.ActivationFunctionType.Sqrt,
                                         bias=eps_sb[:NGT])
                    nc.vector.reciprocal(out=gst[:, 1:2], in_=tmp[:, 1:2])
                    # broadcast back to channels: [mean_c, rstd_c]
                    bps = ps.tile([128, 512], F32, tag="ps")
                    nc.tensor.matmul(bps[:, :2], gAT, gst, start=True, stop=True)
                    # cs = rstd*scale ; cb = bias - mean*cs
                    cscb = small.tile([128, 2], F32, tag="cscb")
                    nc.vector.tensor_mul(cscb[:, 0:1], bps[:, 1:2], gnsc[:, t:t + 1])
                    nc.vector.tensor_mul(cscb[:, 1:2], bps[:, 0:1], cscb[:, 0:1])
                    nc.vector.tensor_tensor(cscb[:, 1:2], gnbi[:, t:t + 1], cscb[:, 1:2],
                                            mybir.AluOpType.subtract)
                    nc.vector.tensor_scalar(
                        out=h_sb[:, t, :], in0=x_sb[:, t, :],
                        scalar1=cscb[:, 0:1], scalar2=cscb[:, 1:2],
                        op0=mybir.AluOpType.mult, op1=mybir.AluOpType.add,
                    )

                # ---- q, k projections: q[o, n] = sum_c wqT[c, o] h[c, n] + bq ----
                q_sb = qk.tile([128, CT, NSP], F32R, tag="q")
                k_sb = qk.tile([128, CT, NSP], F32R, tag="k")
                for wT, bias, dst in ((wqT, bq, q_sb), (wkT, bk, k_sb)):
                    for ot in range(CT):
                        for nch in range(2):
                            pp = ps.tile([128, 512], F32, tag="ps")
                            for ct in range(CT):
                                nc.tensor.matmul(
                                    pp,
                                    wT[:, ct, ot * 128:(ot + 1) * 128],
                                    h_sb[:, ct, nch * 512:(nch + 1) * 512],
                                    start=(ct == 0), stop=(ct == CT - 1),
                                )
                            nc.scalar.activation(
                                out=dst[:, ot, nch * 512:(nch + 1) * 512], in_=pp,
                                func=mybir.ActivationFunctionType.Identity,
                                bias=bias[:, ot:ot + 1],
                            )

                # ---- v' = (wo@wv) h, transposed, with ones column ----
                vt = []
                for mt in range(8):
                    v_t = vp.tile([128, 258], F32R, tag="vt")
                    pp = ps.tile([128, 512], F32, tag="ps")
                    for ct in range(CT):
                        nc.tensor.matmul(
                            pp[:, :256],
                            h_sb[:, ct, mt * 128:(mt + 1) * 128],
                            wvpT[:, ct, :],
                            start=(ct == 0), stop=(ct == CT - 1),
                        )
                    nc.scalar.activation(out=v_t[:, :256], in_=pp[:, :256],
                                         func=mybir.ActivationFunctionType.Copy)
                    nc.vector.tensor_copy(out=v_t[:, 256:258], in_=ones)
                    vt.append(v_t)

                # ---- scores (transposed) + exp: E[m, n] = exp(s[n, m]) ----
                et = []
                for mt in range(8):
                    e_t = ep.tile([128, NSP], F32R, tag="et")
                    for nch in range(2):
                        pp = ps.tile([128, 512], F32, tag="ps")
                        for ct in range(CT):
                            nc.tensor.matmul(
                                pp,
                                k_sb[:, ct, mt * 128:(mt + 1) * 128],
                                q_sb[:, ct, nch * 512:(nch + 1) * 512],
                                start=(ct == 0), stop=(ct == CT - 1),
                            )
                        nc.scalar.activation(
                            out=e_t[:, nch * 512:(nch + 1) * 512], in_=pp,
                            func=mybir.ActivationFunctionType.Exp, scale=SM_SCALE,
                        )
                    et.append(e_t)

                # ---- U'[n, co] = sum_m E[m, n] v'[m, co]; col 256 = denom ----
                # ---- then epilogue: transpose, +bo', +x, store ----
                out_sb = outp.tile([128, CT, NSP], F32, tag="osb")
                for nb in range(8):
                    up = ps.tile([128, 512], F32, tag="ps")
                    for mt in range(8):
                        nc.tensor.matmul(
                            up[:, :258],
                            et[mt][:, nb * 128:(nb + 1) * 128],
                            vt[mt],
                            start=(mt == 0), stop=(mt == 7),
                        )
                    rec = small.tile([128, 1], F32, tag="rec")
                    nc.vector.reciprocal(out=rec, in_=up[:, 256:257])
                    o_t = op.tile([128, 256], F32, tag="ot")
                    nc.vector.tensor_scalar_mul(o_t, up[:, :256], rec)
                    for ctb in range(CT):
                        tp = ps.tile([128, 512], F32, tag="ps")
                        nc.tensor.transpose(tp[:, :128], o_t[:, ctb * 128:(ctb + 1) * 128], ident)
                        seg = out_sb[:, ctb, nb * 128:(nb + 1) * 128]
                        nc.scalar.activation(out=seg, in_=tp[:, :128],
                                             func=mybir.ActivationFunctionType.Identity,
                                             bias=bop[:, ctb:ctb + 1])
                        nc.vector.tensor_add(seg, seg, x_sb[:, ctb, nb * 128:(nb + 1) * 128])

                nc.sync.dma_start(out=out_d[b].rearrange("(t p) n -> p t n", p=128),
                                  in_=out_sb)

    nc.compile()
    return nc


def _prep(inputs):
    f64 = np.float64
    wq = np.asarray(inputs["wq"], f64)
    wk = np.asarray(inputs["wk"], f64)
    wv = np.asarray(inputs["wv"], f64)
    wo = np.asarray(inputs["wo"], f64)
    bv = np.asarray(inputs["bv"], f64)
    bo = np.asarray(inputs["bo"], f64)

    gmat = np.zeros((128, NGT), np.float32)
    gmat[np.arange(128), np.arange(128) // GS] = 1.0

    common = {
        "wqT": np.ascontiguousarray(wq.T, np.float32),
        "wkT": np.ascontiguousarray(wk.T, np.float32),
        "wvpT": np.ascontiguousarray((wo @ wv).T, np.float32),
        "bq": np.ascontiguousarray(inputs["bq"], np.float32),
        "bk": np.ascontiguousarray(inputs["bk"], np.float32),
        "bop": np.ascontiguousarray(wo @ bv + bo, np.float32),
        "gnsc": np.ascontiguousarray(inputs["gn_scale"], np.float32),
        "gnbi": np.ascontiguousarray(inputs["gn_bias"], np.float32),
        "gmatA": gmat,
        "gmatAT": np.ascontiguousarray(gmat.T),
        "ident": np.eye(128, dtype=np.float32),
    }
    x = np.asarray(inputs["x"], np.float32).reshape(B, C, NSP)
    in_maps = []
    for i in range(N_CORES):
        m = dict(common)
        m["x"] = np.ascontiguousarray(x[i * BL:(i + 1) * BL])
        in_maps.append(m)
    return in_maps


def _run(inputs, **spmd_kwargs):
    if "nc" not in _CACHE:
        _CACHE["nc"] = _build()
    nc = _CACHE["nc"]
    in_maps = _prep(inputs)
    res = run_bass_kernel_spmd(nc, in_maps, core_ids=list(range(N_CORES)), **spmd_kwargs)
    out = np.concatenate([r["out"] for r in res.results], axis=0)
    return out.reshape(B, C, H, W).astype(np.float32), res


def kernel(**inputs) -> np.ndarray:
    out, _ = _run(inputs)
    return out


# revision 16
# speedup vs baseline: 1.0304x; 1.0304x over previous
"""AttnBlock (GroupNorm -> 1x1-conv QKV self-attention -> 1x1-conv out -> residual)
for Trainium2, data-parallel over batch across 8 NeuronCores.

Contract: kernel(**inputs) takes the FULL inputs (np arrays, dtypes as in
setup_inputs) and returns the FULL output [32, 256, 32, 32] fp32.

Math (per batch, all folds exact in real arithmetic, done in fp64 on host):
  h = GroupNorm(x)                                  [C, N]
  scores s[n,m] = (q_n + bq) . (k_m + bk) / 16  with q = wq h, k = wk h
    = (h_n^T M h_m + gam . h_n + w2 . h_m + c2) / 16,
      M = wq^T wk, gam = wq^T bk, w2 = wk^T bq, c2 = bq.bk
  softmax over m; o = attn @ v; out = x + wo o + bo
    wo folded: v' = (wo wv) h, out = x + (v' P^T) + (wo bv + bo)
  Softmax denominators come free from a ones-column appended to the v'
  tiles; no max-subtraction is needed (|s| <= ~9 here, exp is fp32-safe).

Channel layout: c = 2p + j (partition p, slot j in {0,1}) so every x/out DMA
is fully contiguous per partition and each partition's channels belong to a
single group (group g = p // 4, 32 groups -> one stat-reduce mm per batch).
Weight matrices are column-permuted on the host so that every matmul operand
slice on the device is contiguous.

Device dataflow per batch (4 per core):
  g = M^T h + gam (fp32r)      [matmul, ACT Identity w/ bias]
  E[m,n] = exp(s^T) (bf16)     [lhsT=g block, rhs=h chunk; ACT Exp with
                                per-partition bias r2t = (w2.h_m + c2)/16,
                                r2 computed as an extra column of the v' mm]
  v't[m, 0:256] = v' (bf16), [:,256:258] = 1
  U[n, 0:258] = sum_m E[m,nb] v't[m]   (psum);  oT = U[:, :256] / U[:,256]
  out = PE-transpose(oT) + (x + bo')
"""
import numpy as np

import concourse.bacc as bacc
import concourse.mybir as mybir
import concourse.tile as tile
from concourse.bass_utils import run_bass_kernel_spmd

N_CORES = 8
B, C, H, W = 32, 256, 32, 32
NSP = H * W            # 1024 spatial positions
BL = B // N_CORES      # 4 batches per core
CT = 2                 # channel slots per partition (c = 2p + j)
NG = 32                # groups (one per 4 partitions)
GS = 8                 # channels per group
EPS = 1e-5
SM_SCALE = 1.0 / 16.0  # C ** -0.5
F32 = mybir.dt.float32
F32R = mybir.dt.float32r
BF16 = mybir.dt.bfloat16
AF = mybir.ActivationFunctionType
ALU = mybir.AluOpType

# packed const columns: gA | gnsc | gnbi | gam | bop | c2 | ident
PK_GA, PK_SC, PK_BI, PK_GAM, PK_BOP, PK_C2, PK_ID = 0, 32, 34, 36, 38, 40, 41
PK_W = 41 + 128

_CACHE: dict = {}


def _build():
    nc = bacc.Bacc(None, target_bir_lowering=False)

    x_d = nc.dram_tensor("x", [BL, C, NSP], F32, kind="ExternalInput")
    wmT_d = nc.dram_tensor("wmT", [C, C], F32, kind="ExternalInput")
    wvpT_d = nc.dram_tensor("wvpT", [C, 258], F32, kind="ExternalInput")
    pack_d = nc.dram_tensor("cpack", [128, PK_W], F32, kind="ExternalInput")
    out_d = nc.dram_tensor("out", [BL, C, NSP], F32, kind="ExternalOutput")

    with tile.TileContext(nc) as tc:
        with tc.tile_pool(name="consts", bufs=1) as consts, \
             tc.tile_pool(name="xp", bufs=4) as xp, \
             tc.tile_pool(name="hp", bufs=2) as hp, \
             tc.tile_pool(name="gp", bufs=2) as gp, \
             tc.tile_pool(name="vp", bufs=12) as vp, \
             tc.tile_pool(name="ep", bufs=12) as ep, \
             tc.tile_pool(name="op", bufs=8) as op, \
             tc.tile_pool(name="xbp", bufs=2) as xbp, \
             tc.tile_pool(name="outp", bufs=2) as outp, \
             tc.tile_pool(name="small", bufs=6) as small, \
             tc.tile_pool(name="r2p", bufs=12) as r2p, \
             tc.tile_pool(name="cscbp", bufs=8) as cscbp, \
             tc.tile_pool(name="ps1", bufs=4, space="PSUM") as ps1, \
             tc.tile_pool(name="ps2", bufs=2, space="PSUM") as ps2:

            # ---- one packed const DMA, then x[0], weights, x[1..3] ----
            cpack = consts.tile([128, PK_W], F32, tag="cpack")
            nc.sync.dma_start(out=cpack, in_=pack_d[:, :])
            gA = cpack[:, PK_GA:PK_GA + 32]
            gnsc = cpack[:, PK_SC:PK_SC + 2]
            gnbi = cpack[:, PK_BI:PK_BI + 2]
            gam = cpack[:, PK_GAM:PK_GAM + 2]
            bop = cpack[:, PK_BOP:PK_BOP + 2]
            c2t = cpack[:, PK_C2:PK_C2 + 1]
            ident = cpack[:, PK_ID:PK_ID + 128]

            x_tiles = []
            cscb_tiles = {}
            x_sb = xp.tile([128, CT, NSP], F32, tag="x")
            nc.sync.dma_start(out=x_sb, in_=x_d[0].rearrange("(p j) n -> p j n", j=CT))
            x_tiles.append(x_sb)
            wmT = consts.tile([128, CT, C], F32R, tag="wmT")
            nc.sync.dma_start(out=wmT, in_=wmT_d.rearrange("(p j) o -> p j o", j=CT).bitcast(F32R))
            wvpT = consts.tile([128, CT, 258], F32R, tag="wvpT")
            nc.sync.dma_start(out=wvpT, in_=wvpT_d.rearrange("(p j) o -> p j o", j=CT).bitcast(F32R))
            for b in range(1, BL):
                x_sb = xp.tile([128, CT, NSP], F32, tag="x")
                nc.sync.dma_start(out=x_sb, in_=x_d[b].rearrange("(p j) n -> p j n", j=CT))
                x_tiles.append(x_sb)

            ones = consts.tile([128, 2], F32, tag="ones")
            nc.vector.memset(ones, 1.0)
            eps_sb = consts.tile([128, 1], F32, tag="eps")
            nc.vector.memset(eps_sb, EPS)

            # gAT = gA^T via one PE transpose (feeds the stat broadcast mm)
            gat_ps = ps1.tile([128, 512], F32, tag="ps1")
            nc.tensor.transpose(gat_ps[:32, :128], gA, ident)
            gAT = consts.tile([32, 128], F32, tag="gAT")
            nc.vector.tensor_copy(out=gAT, in_=gat_ps[:32, :128])

            def gn_stats_chain(b):
                """bn stats -> one 32-group reduce mm -> rstd via ln/exp ->
                one broadcast mm -> per-channel (cs, cb) [128, 2+2]."""
                x_sb = x_tiles[b]
                mvs = []
                for j in range(CT):
                    st = small.tile([128, 2, 6], F32, tag="bnst")
                    nc.vector.bn_stats(out=st[:, 0, :], in_=x_sb[:, j, 0:512])
                    nc.vector.bn_stats(out=st[:, 1, :], in_=x_sb[:, j, 512:1024])
                    mv = small.tile([128, 2], F32, tag="mv")
                    nc.vector.bn_aggr(out=mv, in_=st)
                    mvs.append(mv)
                # msum = [sum_j mean_j, sum_j (var_j + mean_j^2)]
                msum = small.tile([128, 2], F32, tag="msum")
                m2 = small.tile([128, 2], F32, tag="m2")
                for j in range(CT):
                    nc.vector.tensor_mul(m2[:, j:j + 1], mvs[j][:, 0:1], mvs[j][:, 0:1])
                    nc.vector.tensor_add(m2[:, j:j + 1], m2[:, j:j + 1], mvs[j][:, 1:2])
                nc.vector.tensor_add(msum[:, 0:1], mvs[0][:, 0:1], mvs[1][:, 0:1])
                nc.vector.tensor_add(msum[:, 1:2], m2[:, 0:1], m2[:, 1:2])
                gnps = ps1.tile([128, 512], F32, tag="ps1")
                nc.tensor.matmul(gnps[:NG, :2], gA, msum, start=True, stop=True)
                gst = small.tile([NG, 2], F32, tag="gst")
                nc.scalar.mul(out=gst, in_=gnps[:NG, :2], mul=1.0 / GS)
                tmp = small.tile([NG, 2], F32, tag="gtmp")
                nc.vector.tensor_mul(tmp[:, 0:1], gst[:, 0:1], gst[:, 0:1])
                nc.vector.tensor_tensor(tmp[:, 1:2], gst[:, 1:2], tmp[:, 0:1],
                                        ALU.subtract)
                # rstd = exp(-0.5*ln(var+eps)): keeps ACT on one table set
                nc.scalar.activation(out=tmp[:, 1:2], in_=tmp[:, 1:2],
                                     func=AF.Ln, bias=eps_sb[:NG])
                nc.scalar.activation(out=gst[:, 1:2], in_=tmp[:, 1:2],
                                     func=AF.Exp, scale=-0.5)
                gbps = ps1.tile([128, 512], F32, tag="ps1")
                nc.tensor.matmul(gbps[:, :2], gAT, gst, start=True, stop=True)
                # cs[:, j] = rstd * gnsc[:, j]; cb[:, j] = gnbi[:, j] - mean*cs
                cscb = cscbp.tile([128, 4], F32, tag="cscb")
                nc.vector.tensor_scalar_mul(cscb[:, 0:2], gnsc, gbps[:, 1:2])
                nc.vector.tensor_scalar_mul(cscb[:, 2:4], cscb[:, 0:2], gbps[:, 0:1])
                nc.vector.tensor_tensor(cscb[:, 2:4], gnbi, cscb[:, 2:4], ALU.subtract)
                cscb_tiles[b] = cscb

            gn_stats_chain(0)

            # ---- per-batch attention pipeline ----
            for b in range(BL):
                if b + 1 < BL:
                    gn_stats_chain(b + 1)
                x_sb = x_tiles[b]
                h_sb = hp.tile([128, CT, NSP], F32R, tag="h")
                cscb = cscb_tiles[b]
                for j in range(CT):
                    nc.vector.tensor_scalar(
                        out=h_sb[:, j, :], in0=x_sb[:, j, :],
                        scalar1=cscb[:, j:j + 1], scalar2=cscb[:, 2 + j:3 + j],
                        op0=ALU.mult, op1=ALU.add,
                    )

                # ---- xb = x + bo' (residual + folded out-bias) ----
                xb_sb = xbp.tile([128, CT, NSP], F32, tag="xb")
                for j in range(CT):
                    nc.vector.tensor_scalar_add(
                        out=xb_sb[:, j, :], in0=x_sb[:, j, :],
                        scalar1=bop[:, j:j + 1],
                    )

                # ---- g = M^T h + gam: wmT cols are packed [ot][q] so the
                #      lhsT slice for output slot ot is contiguous ----
                g_sb = gp.tile([128, CT, NSP], F32R, tag="g")
                for ot in range(CT):
                    gpp = ps2.tile([128, 1024], F32, tag="ps2")
                    for nch in range(2):
                        for ct in range(CT):
                            nc.tensor.matmul(
                                gpp[:, nch * 512:(nch + 1) * 512],
                                wmT[:, ct, ot * 128:(ot + 1) * 128],
                                h_sb[:, ct, nch * 512:(nch + 1) * 512],
                                start=(ct == 0), stop=(ct == CT - 1),
                            )
                    nc.scalar.activation(out=g_sb[:, ot, :], in_=gpp,
                                         func=AF.Identity, bias=gam[:, ot:ot + 1])

                # ---- v' (transposed, bf16) + r2t from the extra column ----
                vt = []
                r2t = []
                for mt in range(8):
                    v_t = vp.tile([128, 258], BF16, tag="vt")
                    vpp = ps1.tile([128, 512], F32, tag="ps1")
                    for ct in range(CT):
                        nc.tensor.matmul(
                            vpp[:, :258],
                            h_sb[:, ct, mt * 128:(mt + 1) * 128],
                            wvpT[:, ct, :],
                            start=(ct == 0), stop=(ct == CT - 1),
                        )
                    nc.scalar.activation(out=v_t[:, :256], in_=vpp[:, :256],
                                         func=AF.Copy)
                    r2 = r2p.tile([128, 1], F32, tag="r2")
                    nc.vector.tensor_tensor(r2, vpp[:, 256:257], c2t, ALU.add)
                    nc.vector.tensor_copy(out=v_t[:, 256:258], in_=ones)
                    vt.append(v_t)
                    r2t.append(r2)

                # ---- scores (transposed) + exp:
                #      E[m, n] = exp((g_m . h_n)/16 + r2t[m]) in bf16 ----
                # contraction runs over g's output channels: g slot ct holds
                # co = 2q + ct, matching h slot ct channels 2p + ct... the
                # contraction must pair g[c, m] with h[c, n] over the SAME c:
                # both operands' slot-ct tiles hold channels {2i + ct}.
                et = []
                for mt in range(8):
                    e_t = ep.tile([128, NSP], BF16, tag="et")
                    spp = ps2.tile([128, 1024], F32, tag="ps2")
                    for nch in range(2):
                        for ct in range(CT):
                            nc.tensor.matmul(
                                spp[:, nch * 512:(nch + 1) * 512],
                                g_sb[:, ct, mt * 128:(mt + 1) * 128],
                                h_sb[:, ct, nch * 512:(nch + 1) * 512],
                                start=(ct == 0), stop=(ct == CT - 1),
                            )
                    nc.scalar.activation(out=e_t, in_=spp, func=AF.Exp,
                                         scale=SM_SCALE, bias=r2t[mt])
                    et.append(e_t)

                # ---- U[n, :258] = sum_m E[m, nblock] v't[m]; normalize ----
                ot_tiles = []
                for nb in range(8):
                    up = ps1.tile([128, 512], F32, tag="ps1")
                    for mt in range(8):
                        nc.tensor.matmul(
                            up[:, :258],
                            et[mt][:, nb * 128:(nb + 1) * 128],
                            vt[mt],
                            start=(mt == 0), stop=(mt == 7),
                        )
                    rec = small.tile([128, 1], F32, tag="rec")
                    nc.vector.reciprocal(out=rec, in_=up[:, 256:257])
                    o_t = op.tile([128, 256], F32, tag="ot")
                    nc.vector.tensor_scalar_mul(o_t, up[:, :256], rec)
                    ot_tiles.append(o_t)

                # ---- epilogue: PE transposes + (x + bo') add ----
                # wvpT's 256 value columns are packed [j][q], so o_t's cols
                # 128j..128j+128 transpose into out partitions for slot j
                out_sb = outp.tile([128, CT, NSP], F32, tag="osb")
                for j in range(CT):
                    for nb in range(8):
                        tp = ps1.tile([128, 512], F32, tag="ps1")
                        nc.tensor.transpose(
                            tp[:, :128],
                            ot_tiles[nb][:, j * 128:(j + 1) * 128],
                            ident,
                        )
                        seg = out_sb[:, j, nb * 128:(nb + 1) * 128]
                        nc.vector.tensor_tensor(
                            seg, tp[:, :128],
                            xb_sb[:, j, nb * 128:(nb + 1) * 128],
                            ALU.add)

                nc.sync.dma_start(out=out_d[b].rearrange("(p j) n -> p j n", j=CT),
                                  in_=out_sb)

    nc.compile()
    return nc


def _col_pack(a):
    """Permute columns of [R, 256] so cols become [j][q] with co = 2q + j."""
    return a.reshape(a.shape[0], 128, 2).transpose(0, 2, 1).reshape(a.shape[0], 256)


def _prep(inputs):
    f64 = np.float64
    wq = np.asarray(inputs["wq"], f64)
    wk = np.asarray(inputs["wk"], f64)
    wv = np.asarray(inputs["wv"], f64)
    wo = np.asarray(inputs["wo"], f64)
    bq = np.asarray(inputs["bq"], f64)
    bk = np.asarray(inputs["bk"], f64)
    bv = np.asarray(inputs["bv"], f64)
    bo = np.asarray(inputs["bo"], f64)

    # wvpT: [C, 258]: cols 0:256 = (wo wv)^T col-packed, col 256 = (wk^T bq)/16
    wvpT = np.zeros((C, 258), np.float64)
    wvpT[:, :256] = _col_pack((wo @ wv).T)
    wvpT[:, 256] = (wk.T @ bq) * SM_SCALE

    pack = np.zeros((128, PK_W), np.float32)
    pack[np.arange(128), PK_GA + np.arange(128) // 4] = 1.0      # gA
    pack[:, PK_SC:PK_SC + 2] = np.asarray(inputs["gn_scale"], np.float32).reshape(128, 2)
    pack[:, PK_BI:PK_BI + 2] = np.asarray(inputs["gn_bias"], np.float32).reshape(128, 2)
    pack[:, PK_GAM:PK_GAM + 2] = (wq.T @ bk).astype(np.float32).reshape(128, 2)
    pack[:, PK_BOP:PK_BOP + 2] = (wo @ bv + bo).astype(np.float32).reshape(128, 2)
    pack[:, PK_C2] = np.float32(float(bq @ bk) * SM_SCALE)
    pack[:, PK_ID:PK_ID + 128] = np.eye(128, dtype=np.float32)

    common = {
        "wmT": np.ascontiguousarray(_col_pack(wk.T @ wq), np.float32),
        "wvpT": np.ascontiguousarray(wvpT, np.float32),
        "cpack": pack,
    }
    x = np.asarray(inputs["x"], np.float32).reshape(B, C, NSP)
    in_maps = []
    for i in range(N_CORES):
        m = dict(common)
        m["x"] = np.ascontiguousarray(x[i * BL:(i + 1) * BL])
        in_maps.append(m)
    return in_maps


def _run(inputs, **spmd_kwargs):
    if "nc" not in _CACHE:
        _CACHE["nc"] = _build()
    nc = _CACHE["nc"]
    in_maps = _prep(inputs)
    res = run_bass_kernel_spmd(nc, in_maps, core_ids=list(range(N_CORES)), **spmd_kwargs)
    out = np.concatenate([r["out"] for r in res.results], axis=0)
    return out.reshape(B, C, H, W).astype(np.float32), res


def kernel(**inputs) -> np.ndarray:
    out, _ = _run(inputs)
    return out
